# revision 55
# baseline (speedup 1.0000x reference)
"""DifferentialAttention Trainium2 kernel (8 NeuronCores, SPMD).

Sharding (per spec hint): data-parallel over the 2 batches x tensor-parallel
over head groups. Core c owns batch c//4 and heads 4r..4r+3 (r = c%4), with
ALL 2048 tokens of its batch resident. This kills the big K/V AllGathers of
the token-sharded layout: attention reads K/V straight from SBUF.

Cross-core traffic that remains:
  - LN statistics: each core has only 512 of the 2048 q/k channels, so the
    per-token variance is a partial sum -> one 16 KB AllReduce.
  - Output exchange: out-proj needs all 16 heads, so attention outputs are
    AllGathered in 8 per-(head-pair, span) slices of 0.5 MB as their
    diff-combines complete; all but the last hide under attention compute.
    The out-projection is then D-sharded (each core computes output channels
    256r..256r+255 for ALL tokens of its batch, with its 256-column slice of
    Wo supplied as input), which keeps the gathered-data readback identical
    on every core - the only rank-dependent pieces are host-side input
    slices and output stitching.

Layout strategy (kept from the token-sharded version): features on
partitions, tokens on free dim, so the chain proj -> scores -> AV -> out-proj
needs zero on-device transposes.
  - LN mean subtraction is folded into host-side column-centering of the
    (ternary-quantized) weights; variance comes from a matmul of squared
    activations against a replicated 1/(CH*gamma^2) stationary (partial, then
    AllReduced).
  - K's 1/std never touches K: it rides as the per-key `scale` operand of the
    softmax exp (fp32 now - no fp16 hi/lo split needed since it stays local).
  - softmax runs without max-subtraction; the denominator is a 65th "ones"
    column appended to V.
  - softmax denominators are reshaped [1,512]->[4,128] through DRAM before
    the DVE reciprocal (reciprocal cost is free-dim-length * 8 cycles,
    independent of partition count - the flat [1,512] form was 4x slower).

Attention iterates 16x (head-pair, branch, query-span); each iteration's
score matmuls + exps interleave with the previous iteration's AV matmuls so
the in-order PE queue never waits on the ScalarE exp stream.
"""

import os
import sys
import types

for _p in ("/opt/trn_rl_repo",):
    if os.path.isdir(_p) and _p not in sys.path:
        sys.path.append(_p)

import numpy as np

import concourse.bass as bass
import concourse.tile as tile
from concourse.bass import _add_dep_helper
from concourse import bacc, mybir
from concourse.bass_utils import run_bass_kernel_spmd


def _install_ntff_shim():
    """bass_utils imports antenv.axon_hooks when tracing under axon; the
    container antenv stub lacks it. Back it with the ctypes hook."""
    if "antenv.axon_hooks" in sys.modules:
        return
    try:
        from trn_agent_boot.trn_boot import _ntff_profile_via_ctypes

        hook = _ntff_profile_via_ctypes("/opt/axon/libaxon_pjrt.so")
    except Exception:
        hook = None
    mod = types.ModuleType("antenv.axon_hooks")
    mod.get_axon_ntff_profile_hook = lambda: hook
    sys.modules["antenv.axon_hooks"] = mod


_install_ntff_shim()

# ----- problem dims (hardcoded per spec) -----
B, T, D = 2, 2048, 1024
H, DH = 16, 64
CH = 2 * H * DH  # 2048
EPS = 1e-5
NCORES = 8
GS = 4  # cores per batch group
GROUPS = [[0, 1, 2, 3], [4, 5, 6, 7]]

HL = H // GS  # 4 local heads
HPL = HL // 2  # 2 local head pairs
QKC = 2 * HL * DH  # 512 local q/k channels
VC = HL * DH  # 256 local v channels
KT = D // 128  # 8 contraction tiles for projections
CT = QKC // 128  # 4 local qk channel tiles
NSP = 4  # query spans per batch
SP = T // NSP  # 512
NKC = T // 128  # 16 key chunks
TLOC = T // GS  # 512 output tokens per core

F32 = mybir.dt.float32
MM_DT = "f16"  # "f16" | "bf16" | "f32r"
_DT_MAP = {
    "f16": mybir.dt.float16,
    "bf16": mybir.dt.bfloat16,
    "f32r": mybir.dt.float32r,
}

_PROG_CACHE: dict = {}


def _bcast_part(ap, n):
    """AP view replicating a 1-partition AP across n partitions (step 0)."""
    return bass.AP(tensor=ap.tensor, offset=ap.offset, ap=[[0, n]] + list(ap.ap)[1:])


def build_program(mm_dt=MM_DT):
    DT = _DT_MAP[mm_dt]

    nc = bacc.Bacc("TRN2", target_bir_lowering=False, debug=False, num_devices=NCORES)

    xT = nc.dram_tensor("xT", [D, T], DT, kind="ExternalInput").ap()
    wq_t = nc.dram_tensor("wq_t", [D, QKC], DT, kind="ExternalInput").ap()
    wk_t = nc.dram_tensor("wk_t", [D, QKC], DT, kind="ExternalInput").ap()
    wv_t = nc.dram_tensor("wv_t", [D, VC], DT, kind="ExternalInput").ap()
    wo_t = nc.dram_tensor("wo_t", [D, VC], DT, kind="ExternalInput").ap()
    wsq_q = nc.dram_tensor("wsq_q", [128, QKC], DT, kind="ExternalInput").ap()
    wsq_k = nc.dram_tensor("wsq_k", [128, QKC], DT, kind="ExternalInput").ap()
    lam_in = nc.dram_tensor("lam", [1, 1], F32, kind="ExternalInput").ap()
    ones_one_in = nc.dram_tensor("ones_one", [128, HL], DT, kind="ExternalInput").ap()
    one11_in = nc.dram_tensor("one11", [1, 1], F32, kind="ExternalInput").ap()
    eps11_in = nc.dram_tensor("eps11", [1, 1], F32, kind="ExternalInput").ap()
    id128_in = nc.dram_tensor("id128", [128, 128], DT, kind="ExternalInput").ap()
    yT = nc.dram_tensor("yT", [VC, T], F32, kind="ExternalOutput").ap()

    with tile.TileContext(nc) as tc:
        with (
            tc.tile_pool(name="const", bufs=1) as const,
            tc.tile_pool(name="dram", bufs=1, space="DRAM") as dram,
            tc.tile_pool(name="dden_pool", bufs=8, space="DRAM") as dden_pool,
            tc.tile_pool(name="qf_p", bufs=1) as qf_p,
            tc.tile_pool(name="kf_p", bufs=1) as kf_p,
            tc.tile_pool(name="vh_p", bufs=1) as vh_p,
            tc.tile_pool(name="attn_p", bufs=1) as attn_p,
            tc.tile_pool(name="rk_p", bufs=1) as rk_p,
        ):
            # constants + tiny inputs
            ones_one = const.tile([128, HL], DT)
            nc.scalar.dma_start(ones_one[:], ones_one_in[:])
            lam_sb = const.tile([8, 1], F32)
            nc.scalar.dma_start(lam_sb[:], _bcast_part(lam_in[:], 8))
            wsq_q_sb = const.tile([128, QKC], DT)
            nc.scalar.dma_start(wsq_q_sb[:], wsq_q[:])
            wsq_k_sb = const.tile([128, QKC], DT)
            nc.scalar.dma_start(wsq_k_sb[:], wsq_k[:])
            one11_sb = const.tile([1, 1], F32)
            nc.scalar.dma_start(one11_sb[:], one11_in[:])
            eps_sb = const.tile([128, 1], F32)
            nc.scalar.dma_start(eps_sb[:], _bcast_part(eps11_in[:], 128))
            id128_sb = const.tile([128, 128], DT)
            nc.scalar.dma_start(id128_sb[:], id128_in[:])
            # preload the natural_log_exp table set at t~0 so neither the
            # rstd chain (ln+exp) nor the attention exps pay a table switch
            dummy_act = const.tile([1, 1], F32)
            nc.scalar.activation(
                dummy_act[:], one11_sb[:], mybir.ActivationFunctionType.Ln
            )
            nc.scalar.activation(
                dummy_act[:], dummy_act[:], mybir.ActivationFunctionType.Exp
            )

            # persistent SBUF state
            qf_sb = qf_p.tile([128, CT, T], DT)  # LN'd+scaled q
            kf_sb = kf_p.tile([128, CT, T], DT)  # raw centered k
            vh_sb = vh_p.tile([128, NKC, HL, 65], DT)  # v + ones col
            attn_sb = attn_p.tile([128, HPL, T], DT)
            rk_sb = rk_p.tile([128, 2, NKC], F32)  # [:,0]=rstd_q, [:,1]=rstd_k

            # DRAM buffers for collectives. The stats payload is 16 KB but
            # sub-~100KB collectives hit a ~50us slow path, so pad to 128 KB
            # (columns 32.. are never written or read). Layout is [128, 32]
            # with token = chunk*128 + partition so every DMA stays
            # contiguous per partition.
            SCOLS = 256
            stats_in = dram.tile([128, SCOLS], F32)
            stats_out = dram.tile([GS, 128, SCOLS], F32)
            warm_in = dram.tile([128, SCOLS], DT)
            warm_out = dram.tile([GS, 128, SCOLS], DT)
            rq_dram = dram.tile([1, T], F32)
            ag_in = [
                [dram.tile([128, SP], DT, name=f"ag_in{h}_{s}") for s in range(NSP)]
                for h in range(HPL)
            ]
            ag_out = [
                [
                    dram.tile([GS, 128, SP], DT, name=f"ag_out{h}_{s}")
                    for s in range(NSP)
                ]
                for h in range(HPL)
            ]

            # ---------------- Phase 1: projections + LN stats -------------
            with (
                tc.tile_pool(name="xp", bufs=1) as xp,
                tc.tile_pool(name="wk_p", bufs=1) as wk_p,
                tc.tile_pool(name="wq_p", bufs=1) as wq_p,
                tc.tile_pool(name="wv_p", bufs=1) as wv_p,
                tc.tile_pool(name="qraw_p", bufs=1) as qraw_p,
                tc.tile_pool(name="sq_p", bufs=3) as sq_p,
                tc.tile_pool(name="stat_p", bufs=1) as stat_p,
                tc.tile_pool(name="rs_p", bufs=1) as rs_p,
                tc.tile_pool(name="pp", bufs=1, space="PSUM") as pp,
            ):
                # weight strips first (K proj starts as soon as these land),
                # then x halves in consumption order; single big DMAs keep
                # the sync-queue trigger count low (~0.7us per trigger)
                wk_sb = wk_p.tile([128, KT, QKC], DT)
                nc.sync.dma_start(
                    wk_sb[:, :, 0:128],
                    wk_t.rearrange("(j p) c -> p j c", p=128)[:, :, 0:128],
                )
                xT_sb = xp.tile([128, KT, T], DT)
                nc.sync.dma_start(
                    xT_sb[:, :, 0:SP],
                    xT.rearrange("(k p) t -> p k t", p=128)[:, :, 0:SP],
                )
                nc.sync.dma_start(
                    wk_sb[:, :, 128:QKC],
                    wk_t.rearrange("(j p) c -> p j c", p=128)[:, :, 128:QKC],
                )
                for s in range(1, NSP):
                    nc.sync.dma_start(
                        xT_sb[:, :, s * SP : (s + 1) * SP],
                        xT.rearrange("(k p) t -> p k t", p=128)[
                            :, :, s * SP : (s + 1) * SP
                        ],
                    )
                wq_sb = wq_p.tile([128, KT, QKC], DT)
                nc.sync.dma_start(wq_sb[:], wq_t.rearrange("(j p) c -> p j c", p=128))
                wv_sb = wv_p.tile([128, KT, VC], DT)
                nc.sync.dma_start(wv_sb[:], wv_t.rearrange("(j p) c -> p j c", p=128))

                # warm-up collective: absorbs the comm-init barrier and the
                # first-collective penalty while phase 1 computes
                cc_warm = nc.gpsimd.collective_compute(
                    "AllGather",
                    mybir.AluOpType.bypass,
                    replica_groups=GROUPS,
                    ins=[warm_in[:]],
                    outs=[warm_out[:]],
                )

                stats_q = stat_p.tile([1, T], F32, name="stats_q")
                stats_k = stat_p.tile([1, T], F32, name="stats_k")
                stats_rows = (stats_q, stats_k)
                qraw = qraw_p.tile([128, CT, T], DT)

                kvps = pp.tile([128, 2, NKC], F32, tag="kvps")

                def qk_proj(w_sb, wsq_sb, out_row, sink):
                    # sink(ps, t, s) stores the raw projection tile
                    for s in range(NSP):
                        var = pp.tile([128, SP], F32, tag="var", bufs=2, name="var")
                        for t in range(CT):
                            ps = pp.tile([128, SP], F32, tag="proj", bufs=2, name="ps")
                            for j in range(KT):
                                nc.tensor.matmul(
                                    ps[:],
                                    w_sb[:, j, t * 128 : (t + 1) * 128],
                                    xT_sb[:, j, s * SP : (s + 1) * SP],
                                    start=(j == 0),
                                    stop=(j == KT - 1),
                                )
                            sink(ps, t, s)
                            sq = sq_p.tile([128, SP], DT, tag="sq", name="sq")
                            nc.scalar.square(sq[:], ps[:])
                            nc.tensor.matmul(
                                var[:],
                                wsq_sb[:, t * 128 : (t + 1) * 128],
                                sq[:],
                                start=(t == 0),
                                stop=(t == CT - 1),
                            )
                        nc.vector.tensor_copy(
                            stats_rows[out_row][0:1, s * SP : (s + 1) * SP],
                            var[0:1, :],
                        )
                        # transpose this span's stats to token-on-partition
                        # right away via tiny rank-1 matmuls
                        for c in range(4):
                            cc = s * 4 + c
                            nc.tensor.matmul(
                                kvps[:, out_row, cc : cc + 1],
                                stats_rows[out_row][0:1, cc * 128 : (cc + 1) * 128],
                                one11_sb[:],
                                start=True,
                                stop=True,
                            )

                def k_sink(ps, t, s):
                    nc.vector.tensor_copy(kf_sb[:, t, s * SP : (s + 1) * SP], ps[:])

                def q_sink(ps, t, s):
                    nc.vector.tensor_copy(qraw[:, t, s * SP : (s + 1) * SP], ps[:])

                qk_proj(wk_sb, wsq_k_sb, 1, k_sink)
                qk_proj(wq_sb, wsq_q_sb, 0, q_sink)
                # re-pin the ln/exp table while ACT is idle so the rstd chain
                # below doesn't pay the ~1.3us table load on the critical path
                nc.scalar.activation(
                    dummy_act[:], one11_sb[:], mybir.ActivationFunctionType.Ln
                )

                st128 = stat_p.tile([128, 2, NKC], F32, name="st128")
                nc.vector.tensor_copy(st128[:], kvps[:])
                nc.sync.dma_start(stats_in[:, 0 : 2 * NKC], st128[:].rearrange("p r c -> p (r c)"))
                # tiny AllReduce is pathologically slow (~70us for 16KB);
                # AllGather the partials and sum locally instead
                cc_ar = nc.gpsimd.collective_compute(
                    "AllGather",
                    mybir.AluOpType.bypass,
                    replica_groups=GROUPS,
                    ins=[stats_in[:]],
                    outs=[stats_out[:]],
                )
                _add_dep_helper(cc_ar.ins, cc_warm.ins, sync=True, reason="cc order")

                # V projection (overlaps the AllReduce)
                for c in range(NKC):
                    vps = pp.tile([128, VC], F32, tag="vproj", bufs=1, name="vps")
                    for j in range(KT):
                        nc.tensor.matmul(
                            vps[:],
                            xT_sb[:, j, c * 128 : (c + 1) * 128],
                            wv_sb[:, j, :],
                            start=(j == 0),
                            stop=(j == KT - 1),
                        )
                    nc.vector.tensor_copy(
                        vh_sb[:, c, :, 0:64],
                        vps[:].rearrange("p (h d) -> p h d", h=HL),
                    )
                    nc.sync.dma_start(vh_sb[:, c, :, 64:65], ones_one[:])

                # rstd for q (per query) and k (per key), [128, 2, NKC] with
                # token = c*128 + p; sum the 4 gathered partials locally
                rvp = rs_p.tile([128, GS, 2, NKC], F32, tag="rvp")
                nc.sync.dma_start(
                    rvp[:].rearrange("p g r c -> p g (r c)"),
                    stats_out[:, :, 0 : 2 * NKC].rearrange("g p c -> p g c"),
                )
                rv01 = rs_p.tile([128, 2, NKC], F32, tag="rv01")
                nc.vector.tensor_add(rv01[:], rvp[:, 0], rvp[:, 1])
                rv23 = rs_p.tile([128, 2, NKC], F32, tag="rv23")
                nc.vector.tensor_add(rv23[:], rvp[:, 2], rvp[:, 3])
                rv = rs_p.tile([128, 2, NKC], F32, tag="rv")
                nc.vector.tensor_add(rv[:], rv01[:], rv23[:])
                # rstd = exp(-0.5 * ln(var+eps)): ln and exp share one ACT
                # table set (preloaded at t~0), unlike sqrt which would force
                # two table switches on the critical path; eps rides the ln
                # bias operand
                lnv = rs_p.tile([128, 2, NKC], F32, tag="lnv")
                nc.scalar.activation(
                    lnv[:], rv[:], mybir.ActivationFunctionType.Ln, bias=eps_sb[:, 0:1]
                )
                nc.scalar.activation(
                    rk_sb[:], lnv[:], mybir.ActivationFunctionType.Exp, scale=-0.5
                )

                # broadcast rstd_q to [128, T] (token on free dim):
                # transpose back via identity matmuls, bounce through DRAM,
                # pipelined per span so the first STT fires early
                rq16 = rs_p.tile([128, NKC], DT, tag="rq16")
                nc.vector.tensor_copy(rq16[:], rk_sb[:, 0, :])
                rqf_sb = rs_p.tile([1, T], DT, tag="rqf_sb")
                rqb_s = []
                for rnd in range(4):
                    rqf_ps = pp.tile([1, SP], F32, tag="rqf", bufs=2, name="rqf")
                    for cc in range(4):
                        nc.tensor.matmul(
                            rqf_ps[0:1, cc * 128 : (cc + 1) * 128],
                            rq16[:, rnd * 4 + cc : rnd * 4 + cc + 1],
                            id128_sb[:],
                            start=True,
                            stop=True,
                        )
                    nc.vector.tensor_copy(
                        rqf_sb[0:1, rnd * SP : (rnd + 1) * SP], rqf_ps[:]
                    )
                    rqb = rs_p.tile([128, SP], DT, tag="rqb", bufs=NSP, name="rqb")
                    nc.gpsimd.partition_broadcast(
                        rqb[:], rqf_sb[0:1, rnd * SP : (rnd + 1) * SP]
                    )
                    rqb_s.append(rqb)
                for tpair in ((0, 2), (1, 3)):
                    for s in range(NSP):  # attention consumption order
                        for t in tpair:
                            nc.vector.scalar_tensor_tensor(
                                out=qf_sb[:, t, s * SP : (s + 1) * SP],
                                in0=qraw[:, t, s * SP : (s + 1) * SP],
                                scalar=DH**-0.5,
                                in1=rqb_s[s][:],
                                op0=mybir.AluOpType.mult,
                                op1=mybir.AluOpType.mult,
                            )

            # ---------------- Phase 2: attention --------------------------
            with (
                tc.tile_pool(name="wo_p", bufs=1) as wo_p,
                tc.tile_pool(name="ye_p", bufs=3) as ye_p,
                tc.tile_pool(name="oT_p", bufs=1) as oT_p,
            ):
                wo_sb = wo_p.tile([128, KT, VC], DT)
                for j in range(KT):
                    nc.sync.dma_start(wo_sb[:, j, :], wo_t[j * 128 : (j + 1) * 128, :])
                oT_sb = oT_p.tile([128, KT, T], DT)
                ccs = [cc_ar]
                from contextlib import ExitStack

                attn_ctx = ExitStack()
                pt_p = attn_ctx.enter_context(tc.tile_pool(name="pt_p", bufs=2 * NKC + 2))
                o1_p = attn_ctx.enter_context(tc.tile_pool(name="o1_p", bufs=NSP))
                rd_p = attn_ctx.enter_context(tc.tile_pool(name="rd_p", bufs=4))
                rdb_p = attn_ctx.enter_context(tc.tile_pool(name="rdb_p", bufs=4))
                scp = attn_ctx.enter_context(tc.tile_pool(name="scp", bufs=2, space="PSUM"))
                avp = attn_ctx.enter_context(tc.tile_pool(name="avp", bufs=4, space="PSUM"))

                o1_tiles = {}

                def stage_ag(hpp, ss):
                    nc.sync.dma_start(
                        ag_in[hpp][ss][:], attn_sb[:, hpp, ss * SP : (ss + 1) * SP]
                    )
                    cc = nc.gpsimd.collective_compute(
                        "AllGather",
                        mybir.AluOpType.bypass,
                        replica_groups=GROUPS,
                        ins=[ag_in[hpp][ss][:]],
                        outs=[ag_out[hpp][ss][:]],
                    )
                    ccs.append(cc)
                    _add_dep_helper(ccs[-1].ins, ccs[-2].ins, sync=True, reason="cc order")

                def combine(st):
                    hpp, bb, ss, avs = st
                    tok = slice(ss * SP, (ss + 1) * SP)
                    # both parities' denominators [2,512] -> [8,128] via DRAM
                    # for one short-free-dim reciprocal (recip cost is
                    # free-dim-length * 8 cycles, independent of partitions)
                    dd = dden_pool.tile([2, SP], F32, tag="dd")
                    for parity, av in ((0, avs[0]), (1, avs[1])):
                        rdc = rd_p.tile([1, SP], F32, tag="rdc")
                        nc.vector.tensor_copy(rdc[:], av[64:65, :])
                        nc.sync.dma_start(dd[parity : parity + 1, :], rdc[:])
                    rd8 = rd_p.tile([8, 128], F32, tag="rd8")
                    nc.sync.dma_start(
                        rd8[:], dd[:].rearrange("r (a b) -> (r a) b", b=128)
                    )
                    rd8r = rd_p.tile([8, 128], F32, tag="rd8r")
                    nc.vector.reciprocal(rd8r[:], rd8[:])
                    if bb == 1:
                        nc.vector.tensor_scalar_mul(rd8r[:], rd8r[:], lam_sb[:, 0:1])
                    dd2 = dden_pool.tile([2, SP], F32, tag="dd2")
                    nc.sync.dma_start(
                        dd2[:].rearrange("r (a b) -> (r a) b", b=128), rd8r[:]
                    )
                    rdb = rdb_p.tile([128, SP], F32, tag="rdb")
                    nc.sync.dma_start(rdb[0:64, :], _bcast_part(dd2[0:1, :], 64))
                    nc.sync.dma_start(rdb[64:128, :], _bcast_part(dd2[1:2, :], 64))
                    for parity, av in ((0, avs[0]), (1, avs[1])):
                        rows = slice(parity * 64, parity * 64 + 64)
                        if bb == 0:
                            o1 = o1_tiles[(hpp, ss)]
                            nc.vector.tensor_mul(o1[rows, :], av[0:64, :], rdb[rows, :])
                        else:
                            o1 = o1_tiles[(hpp, ss)]
                            o2 = rdb_p.tile([128, SP], F32, tag="o2")
                            nc.vector.tensor_mul(o2[rows, :], av[0:64, :], rdb[rows, :])
                            nc.vector.tensor_sub(
                                attn_sb[rows, hpp, tok], o1[rows, :], o2[rows, :]
                            )
                    if bb == 1:
                        # this (head-pair, span) is final -> AllGather it
                        stage_ag(hpp, ss)

                # interleave branches within a head-pair so each span's
                # diff-combine (and its AllGather) fires every 2 iterations
                # instead of all clustering at the end of the pair
                iters = [
                    (hp, b, s) for hp in range(HPL) for s in range(NSP) for b in range(2)
                ]
                prev = None  # (hp, b, s, pts)
                for it, (hp, b, s) in enumerate(iters):
                    tt = b * HPL + hp  # local q/k channel tile
                    qE = qf_sb[0:64, tt, s * SP : (s + 1) * SP]
                    qO = qf_sb[64:128, tt, s * SP : (s + 1) * SP]
                    if b == 0:
                        for _s in (s,):
                            if (hp, _s) not in o1_tiles:
                                o1_tiles[(hp, _s)] = o1_p.tile(
                                    [128, SP], F32, tag="o1", name=f"o1_{hp}_{_s}"
                                )
                    pav = None
                    if prev is not None:
                        pav = (
                            avp.tile([128, SP], F32, tag="av", name="pavE"),
                            avp.tile([128, SP], F32, tag="av", name="pavO"),
                        )
                    pts = []
                    for c in range(NKC):
                        sc = scp.tile([128, 2, SP], F32, tag="sc")
                        nc.tensor.matmul(
                            sc[:, 0, :],
                            kf_sb[0:64, tt, c * 128 : (c + 1) * 128],
                            qE,
                            start=True,
                            stop=True,
                        )
                        nc.tensor.matmul(
                            sc[:, 1, :],
                            kf_sb[64:128, tt, c * 128 : (c + 1) * 128],
                            qO,
                            start=True,
                            stop=True,
                        )
                        pt = pt_p.tile([128, 2, SP], DT, tag="pt")
                        nc.scalar.activation(
                            pt[:],
                            sc[:],
                            mybir.ActivationFunctionType.Exp,
                            scale=rk_sb[:, 1, c : c + 1],
                        )
                        pts.append(pt)
                        if prev is not None:
                            php, pb, ps_, ppts = prev
                            ptt = php * 2
                            nc.tensor.matmul(
                                pav[0][0:65, :],
                                vh_sb[:, c, ptt, :],
                                ppts[c][:, 0, :],
                                start=(c == 0),
                                stop=(c == NKC - 1),
                            )
                            nc.tensor.matmul(
                                pav[1][0:65, :],
                                vh_sb[:, c, ptt + 1, :],
                                ppts[c][:, 1, :],
                                start=(c == 0),
                                stop=(c == NKC - 1),
                            )
                    if prev is not None:
                        combine((prev[0], prev[1], prev[2], pav))
                    prev = (hp, b, s, pts)
                # flush the last iteration's AV + combine
                lhp, lb, ls, lpts = prev
                lav = (
                    avp.tile([128, SP], F32, tag="av", name="lavE"),
                    avp.tile([128, SP], F32, tag="av", name="lavO"),
                )
                ltt = lhp * 2
                for c in range(NKC):
                    nc.tensor.matmul(
                        lav[0][0:65, :],
                        vh_sb[:, c, ltt, :],
                        lpts[c][:, 0, :],
                        start=(c == 0),
                        stop=(c == NKC - 1),
                    )
                    nc.tensor.matmul(
                        lav[1][0:65, :],
                        vh_sb[:, c, ltt + 1, :],
                        lpts[c][:, 1, :],
                        start=(c == 0),
                        stop=(c == NKC - 1),
                    )
                combine((lhp, lb, ls, lav))

                # ---------------- Phase 3: output projection --------------
                # readback: g-tile 2j+hp comes from group-rank j's pair hp.
                # PSUM accumulators recycle avp slots (same shape/tag), so
                # early spans can start as soon as their gathers land.
                for s in range(NSP):
                    for hp in range(HPL):
                        nc.sync.dma_start(
                            oT_sb[:, hp : KT : 2, s * SP : (s + 1) * SP],
                            ag_out[hp][s][:].rearrange("j p t -> p j t"),
                        )
                    for dt_ in range(VC // 128):
                        yps = avp.tile([128, SP], F32, tag="av", name=f"yp{s}_{dt_}")
                        for g in range(KT):
                            nc.tensor.matmul(
                                yps[:],
                                wo_sb[:, g, dt_ * 128 : (dt_ + 1) * 128],
                                oT_sb[:, g, s * SP : (s + 1) * SP],
                                start=(g == 0),
                                stop=(g == KT - 1),
                            )
                        ye = ye_p.tile([128, SP], F32, tag="ye")
                        nc.vector.tensor_copy(ye[:], yps[:])
                        nc.sync.dma_start(
                            yT[dt_ * 128 : (dt_ + 1) * 128, s * SP : (s + 1) * SP],
                            ye[:],
                        )
                attn_ctx.close()  # free attention PSUM/SBUF pools

    nc.compile()
    return nc


# ---------------- host-side preparation ----------------


def _quantize(W):
    W = np.asarray(W, dtype=np.float32)
    scale = np.clip(np.abs(W).mean(axis=1, keepdims=True), 1e-5, None)
    wq = np.clip(np.round(W / scale), -1.0, 1.0)
    return (wq * scale).astype(np.float32)


def prepare_inputs(
    x, Wq, Wk, Wv, Wo, lambda_q, lambda_k, qn_gamma, qn_beta, kn_gamma, kn_beta,
    mm_dt=MM_DT,
):
    """Host prep: quantize + center weights, fold gamma, per-core slices."""
    np_dt = mybir.dt.np(_DT_MAP[mm_dt])
    x = np.asarray(x, dtype=np.float32)
    assert x.shape == (B, T, D)
    assert not (np.any(np.asarray(qn_beta)) or np.any(np.asarray(kn_beta))), (
        "nonzero LN beta not supported by this kernel"
    )

    Wq_e = _quantize(Wq)
    Wk_e = _quantize(Wk)
    Wv_e = _quantize(Wv)
    Wo_e = _quantize(Wo)
    # fold LN mean-subtraction into column-centered weights, gamma into rows
    gq = np.asarray(qn_gamma, np.float32)
    gk = np.asarray(kn_gamma, np.float32)
    Wq_c = (Wq_e - Wq_e.mean(axis=0, keepdims=True)) * gq[:, None]
    Wk_c = (Wk_e - Wk_e.mean(axis=0, keepdims=True)) * gk[:, None]

    Wo_T = np.ascontiguousarray(Wo_e.T).astype(np.float32)  # [HDH, D]

    lam = np.clip(
        np.exp(np.asarray(lambda_q).mean() - np.asarray(lambda_k).mean()), 0.1, 2.0
    ).astype(np.float32)

    xT_b = [np.ascontiguousarray(x[b].T).astype(np_dt) for b in range(B)]

    in_maps = []
    for c in range(NCORES):
        b = c // GS
        r = c % GS
        # owned q/k channel rows: branch-0 pair tiles then branch-1 pair tiles
        rows = np.concatenate(
            [
                np.arange(2 * r * 128, (2 * r + 2) * 128),
                np.arange((8 + 2 * r) * 128, (8 + 2 * r + 2) * 128),
            ]
        )

        def wsq_slice(g):
            w = 1.0 / (CH * np.maximum(g[rows], 1e-12) ** 2)
            # [128, CT*128]: column block t carries w[t*128+p] on partition p
            return np.ascontiguousarray(
                np.repeat(w.reshape(CT, 128).T[:, :, None], 128, axis=2).reshape(
                    128, CT * 128
                )
            ).astype(np_dt)

        in_maps.append(
            {
                "xT": xT_b[b],
                "wq_t": np.ascontiguousarray(Wq_c[rows, :].T).astype(np_dt),
                "wk_t": np.ascontiguousarray(Wk_c[rows, :].T).astype(np_dt),
                "wv_t": np.ascontiguousarray(
                    Wv_e[r * VC : (r + 1) * VC, :].T
                ).astype(np_dt),
                "wo_t": np.ascontiguousarray(
                    Wo_T[:, r * VC : (r + 1) * VC]
                ).astype(np_dt),
                "wsq_q": wsq_slice(gq),
                "wsq_k": wsq_slice(gk),
                "lam": lam.reshape(1, 1),
                "ones_one": np.ones((128, HL), np_dt),
                "one11": np.ones((1, 1), np.float32),
                "eps11": np.full((1, 1), EPS, np.float32),
                "id128": np.eye(128, dtype=np_dt),
            }
        )
    return in_maps


def get_program(mm_dt=MM_DT):
    key = (mm_dt,)
    if key not in _PROG_CACHE:
        _PROG_CACHE[key] = build_program(mm_dt)
    return _PROG_CACHE[key]


def run(inputs, trace=False, mm_dt=MM_DT):
    """Run on hardware; returns (full_output, BassKernelResults)."""
    in_maps = prepare_inputs(**inputs, mm_dt=mm_dt)
    nc = get_program(mm_dt)
    res = run_bass_kernel_spmd(nc, in_maps, list(range(NCORES)), trace=trace)
    out = np.empty((B, T, D), dtype=np.float32)
    for c in range(NCORES):
        b = c // GS
        r = c % GS
        out[b, :, r * VC : (r + 1) * VC] = res.results[c]["yT"].T
    return out, res


def kernel(**inputs) -> np.ndarray:
    out, _ = run(inputs, trace=False)
    return out
